# revision 14
# baseline (speedup 1.0000x reference)
"""AbstractBlast v8: packed stage-2 on PE, pipelined strided-partition shuffles.

Rank space globally permuted (partition p <-> rank r: p = rh*16 + c,
r = c*8 + rh; applied host-side to Vt columns / U rows / S index) so both
layout shuffles are per-j / per-c strided-partition SBUF->SBUF DMAs that
pipeline under stages 1/2:
  shuffle1(j): ypall[j::16, :, :] <- y_j[128, T]   (right after y_j copy)
  shuffle2(c): zall [c::16, :, :] <- z_c[128, T]   (right after z_c copy)
y_j / z_c live in small rotating tiles so copies never WAR-block behind
shuffle DMA reads. Stage 2 is 32 dense 128x128 matmuls
(W2_c[q=(rh*16+j), f=(rh'*16+o)] = S[o,j,c*8+rh] * (rh==rh')) computing z
for all 16 o-blocks at once. Bias rides the PSUM->SBUF copies
(scalar.activation bias= / vector.tensor_scalar add). Shuffle1 DMAs on
gpsimd SWDGE, shuffle2 on sync; x loads j-paired, stores o-paired for
8-16KB DMA rows.  x/Vt/Y/W2/U/z in bf16 (PSUM fp32), out fp32.
"""

import ml_dtypes
import numpy as np

import concourse.bass as bass
import concourse.mybir as mybir
from concourse.bass_utils import run_bass_kernel_spmd
from concourse.tile import TileContext

F32 = mybir.dt.float32
BF16 = mybir.dt.bfloat16

B, T, D = 8, 1024, 4096
BIN, BOUT, BSIN, BSOUT, RANK = 16, 16, 256, 256, 128
NBLK = 2
NTOK = T // NBLK  # 512 = one PSUM bank of fp32

_CACHE = {}


def _split_multi_waits(nc):
    n_split = 0
    for fn in nc.m.functions:
        for bb in fn.blocks:
            new_insts = []
            for inst in bb.instructions:
                si = inst.sync_info
                if si is not None and si.on_wait and len(si.on_wait) > 1:
                    waits = list(si.on_wait)
                    for w in waits[:-1]:
                        nop = mybir.InstNoOp(
                            name=f"{inst.name}-wsplit-{n_split}",
                            ins=[],
                            outs=[],
                            engine=inst.engine,
                            sync_info=mybir.SyncInfo(on_wait=[w], on_update=[]),
                        )
                        n_split += 1
                        new_insts.append(nop)
                    inst.sync_info = mybir.SyncInfo(
                        on_wait=[waits[-1]], on_update=list(si.on_update)
                    )
                new_insts.append(inst)
            bb.instructions = new_insts
    return n_split


def _build_kernel(split_waits=True):
    nc = bass.Bass(trn_type="TRN2")
    xt = nc.dram_tensor("xt", [BIN, 128, 2, T], BF16, kind="ExternalInput")
    vt_w = nc.dram_tensor("vt_w", [128, BIN, 2, RANK], BF16, kind="ExternalInput")
    u_w = nc.dram_tensor("u_w", [128, BOUT, BSOUT], BF16, kind="ExternalInput")
    w2_w = nc.dram_tensor("w2_w", [128, 16, 128], BF16, kind="ExternalInput")
    bias_w = nc.dram_tensor("bias_w", [128, BOUT * 2], F32, kind="ExternalInput")
    ot = nc.dram_tensor("ot", [128, BOUT, 2, T], BF16, kind="ExternalOutput")
    add = mybir.AluOpType.add

    with TileContext(nc) as tc:
        with (
            tc.tile_pool(name="wpool", bufs=1) as wpool,
            tc.tile_pool(name="xpool", bufs=4) as xpool,
            tc.tile_pool(name="bigpool", bufs=1) as bigpool,
            tc.tile_pool(name="smpool", bufs=8) as smpool,
            tc.tile_pool(name="opool", bufs=2) as opool,
            tc.tile_pool(name="yzpsum", bufs=4, space="PSUM") as yzpsum,
            tc.tile_pool(name="opsum", bufs=4, space="PSUM") as opsum,
        ):
            vt_t = wpool.tile([128, BIN, 2, RANK], BF16)
            nc.sync.dma_start(out=vt_t[:, :4, :, :], in_=vt_w[:, :4, :, :])
            nc.sync.dma_start(out=vt_t[:, 4:, :, :], in_=vt_w[:, 4:, :, :])
            u_t = wpool.tile([128, BOUT, BSOUT], BF16)
            nc.gpsimd.dma_start(out=u_t, in_=u_w[:, :, :])
            w2_t = wpool.tile([128, 16, 128], BF16)
            nc.gpsimd.dma_start(out=w2_t, in_=w2_w[:, :, :])
            bias_t = wpool.tile([128, BOUT * 2], F32)
            nc.gpsimd.dma_start(out=bias_t, in_=bias_w[:, :])

            # ypall: packed stage-2 input, partition q=(rh*16+j), free (c, t)
            ypall = bigpool.tile([128, 16, T], BF16, tag="ypall")
            # zall: stage-3 input, partition p=(rh*16+c) [perm rank], free (o, t)
            zall = bigpool.tile([128, BOUT, T], BF16, tag="zall")

            # ---- stage 1 + shuffle1 ----
            for j in range(BIN):
                x_t = xpool.tile([128, 2, T], BF16, tag="xt")
                if j % 2 == 0:
                    nc.sync.dma_start(out=x_t, in_=xt[j, :, :, :])
                else:
                    nc.scalar.dma_start(out=x_t, in_=xt[j, :, :, :])
                y_j = smpool.tile([128, T], BF16, tag="yj")
                for blk in range(NBLK):
                    tok = slice(blk * NTOK, (blk + 1) * NTOK)
                    y_ps = yzpsum.tile([128, NTOK], F32, tag="yz")
                    for k in range(2):
                        nc.tensor.matmul(
                            y_ps,
                            vt_t[:, j, k, :],
                            x_t[:, k, tok],
                            start=(k == 0),
                            stop=(k == 1),
                        )
                    if blk == 0:
                        nc.vector.tensor_copy(y_j[:, tok], y_ps)
                    else:
                        nc.scalar.copy(y_j[:, tok], y_ps)
                if j % 2 == 0:
                    nc.scalar.dma_start(out=ypall[j::16, :, :], in_=y_j)
                else:
                    nc.sync.dma_start(out=ypall[j::16, :, :], in_=y_j)

            # ---- stage 2 + shuffle2 ----
            for c in range(16):
                z_c = smpool.tile([128, T], BF16, tag="zc")
                for blk in range(NBLK):
                    tok = slice(blk * NTOK, (blk + 1) * NTOK)
                    z_ps = yzpsum.tile([128, NTOK], F32, tag="yz")
                    nc.tensor.matmul(
                        z_ps,
                        w2_t[:, c, :],
                        ypall[:, c, tok],
                        start=True,
                        stop=True,
                    )
                    if blk == 0:
                        nc.vector.tensor_copy(z_c[:, tok], z_ps)
                    else:
                        nc.scalar.copy(z_c[:, tok], z_ps)
                if c % 2 == 0:
                    nc.sync.dma_start(out=zall[c::16, :, :], in_=z_c)
                else:
                    nc.scalar.dma_start(out=zall[c::16, :, :], in_=z_c)

            # ---- stage 3: out_o = U_o^T z_o + bias ----
            for op in range(4):
                o_sb = opool.tile([128, 4, 2, T], BF16, tag="o")
                for oo in range(4):
                    o = 4 * op + oo
                    for blk in range(NBLK):
                        tok = slice(blk * NTOK, (blk + 1) * NTOK)
                        for h in range(2):
                            o_ps = opsum.tile([128, NTOK], F32, tag="op")
                            nc.tensor.matmul(
                                o_ps,
                                u_t[:, o, 128 * h : 128 * (h + 1)],
                                zall[:, o, tok],
                                start=True,
                                stop=True,
                            )
                            if (blk * 2 + h) % 2 == 0:
                                nc.scalar.activation(
                                    o_sb[:, oo, h, tok],
                                    o_ps,
                                    mybir.ActivationFunctionType.Identity,
                                    bias=bias_t[:, 2 * o + h : 2 * o + h + 1],
                                    scale=1.0,
                                )
                            else:
                                nc.vector.tensor_scalar(
                                    o_sb[:, oo, h, tok],
                                    o_ps,
                                    bias_t[:, 2 * o + h : 2 * o + h + 1],
                                    None,
                                    add,
                                )
                if op % 2 == 0:
                    nc.sync.dma_start(
                        out=ot[:, 4 * op : 4 * op + 4, :, :], in_=o_sb
                    )
                else:
                    nc.scalar.dma_start(
                        out=ot[:, 4 * op : 4 * op + 4, :, :], in_=o_sb
                    )

    if split_waits:
        _split_multi_waits(nc)
    return nc


# partition p <-> rank: rank(p) = (p % 16) * 8 + p // 16
_PERM = np.array([(p % 16) * 8 + p // 16 for p in range(128)])


def _prep_weights(S, U, Vt, bias):
    bf = ml_dtypes.bfloat16
    vt_w = np.ascontiguousarray(
        Vt[:, :, _PERM].reshape(BIN, 2, 128, RANK).transpose(2, 0, 1, 3).astype(bf)
    )
    u_w = np.ascontiguousarray(U.transpose(1, 0, 2)[_PERM].astype(bf))
    # W2[q=(rh*16+j), c, f=(rh'*16+o)] = S[o, j, c*8+rh] if rh == rh'
    S4 = S.reshape(BOUT, BIN, 16, 8)  # [o, j, c, rh]
    w2 = np.zeros((8, BIN, 16, 8, BOUT), dtype=np.float32)  # [rh, j, c, rh', o]
    for rh in range(8):
        w2[rh, :, :, rh, :] = S4[:, :, :, rh].transpose(1, 2, 0)
    w2_w = np.ascontiguousarray(w2.reshape(128, 16, 128).astype(bf))
    bias_w = np.ascontiguousarray(
        bias.reshape(BOUT, 2, 128).transpose(2, 0, 1).reshape(128, BOUT * 2)
    )
    return vt_w, u_w, w2_w, bias_w


def kernel(x, S, U, Vt, bias):
    x = np.asarray(x, dtype=np.float32)
    S = np.asarray(S, dtype=np.float32)
    U = np.asarray(U, dtype=np.float32)
    Vt = np.asarray(Vt, dtype=np.float32)
    bias = np.asarray(bias, dtype=np.float32)

    bf = ml_dtypes.bfloat16
    vt_w, u_w, w2_w, bias_w = _prep_weights(S, U, Vt, bias)

    if "nc" not in _CACHE:
        _CACHE["nc"] = _build_kernel()
    nc = _CACHE["nc"]

    in_maps = []
    for b in range(B):
        xt = np.ascontiguousarray(
            x[b].T.reshape(BIN, 2, 128, T).transpose(0, 2, 1, 3).astype(bf)
        )
        in_maps.append(
            {"xt": xt, "vt_w": vt_w, "u_w": u_w, "w2_w": w2_w, "bias_w": bias_w}
        )

    res = run_bass_kernel_spmd(nc, in_maps, core_ids=list(range(B)))

    out = np.empty((B, T, D), dtype=np.float32)
    for b in range(B):
        o = res.results[b]["ot"].astype(np.float32)  # [128, 16, 2, T]
        out[b] = o.transpose(3, 1, 2, 0).reshape(T, D)
    return out
